# revision 27
# baseline (speedup 1.0000x reference)
"""Masked per-sample MSE loss (duration-predictor loss) on 8 Trainium2 cores.

Math (per the reference):
    mask[i, j]  = j < token_lengths[i]
    diff        = where(mask, pred - log(alignment), 0.0)
    out         = mean_i( sum_j diff[i,j]^2 / token_lengths[i] )

Sharding: data parallel over the batch dim with length-sorted row assignment.
Rows sorted by token_length; sorted rank r -> row-tile t = r // 1024, core
c = r % 8, partition p = (r % 1024) // 8. Tile t's rows all have length <=
W[t]; with S = max_t ceil(W[t]/(t+1)), tile t is treated as covering columns
[0, S*(t+1)).

Host-side packing: each core's pred and align rows are packed into ONE
[128, 20S] f32 DRAM buffer:
  [0,8S):    diag slots  [pd0 ad0 pd1 ad1 pd2 ad2 pd3 ad3], each S wide,
             where pd_t/ad_t = pred/align of tile t, cols [S*t, S*(t+1))
  [8S,14S):  pred rest   [t1[0,S) t2[0,S) t3[0,S) t2[S,2S) t3[S,2S) t3[2S,3S)]
  [14S,20S): align rest  (same region order)
Invalid positions are padded pred=0 / align=1 (ln 1 = 0), so the device
needs NO masks, iota, or length tensors.  The 8 diag slots have uniform
stride S, so ONE 3-dim DMA [p, slot, w] fetches the same column window of
both halves of all four tiles; sorted order lets diag chunks drop an exact
prefix of partitions — pure traffic pruning to ~52% of the dense bytes.
HWDGE descriptor generation is 625 ns serial per DMA, so DMA count is the
scarce resource: 14 loads cover everything, aligns in two big early slabs
and pred-rest in fine chunks so the tail chain stays short.

Device work: Ln(align) in place (ACT), d = pred - ln(align) in place (DVE or
Pool tensor_sub), then per-(piece, tile) square-with-row-sum-accum (DVE
scalar_tensor_tensor in place, ACT Square into scratch for a few big slices)
into a distinct rs column.  Compute always runs on all 128 partitions (the
ISA forbids partition-offset compute); rows below a pruned chunk's p0 leave
garbage in that chunk's rs column, which the host combine ignores.

Per-row divide by length and the global mean run on the host in float64.

Raw Bass with explicit semaphores (the walrus build rejects compute
instructions carrying more than one sync-wait, so waits are standalone)."""

from contextlib import ExitStack

import numpy as np

import concourse.bass as bass
from concourse import mybir
from concourse.bass_utils import run_bass_kernel_spmd

B, T = 4096, 2048
N_CORES = 8
RPC = B // N_CORES
P = 128
N_TILES = RPC // P
GROUP = P * N_CORES

F32 = mybir.dt.float32

_CACHE: dict = {}

# rest regions: (index, tile, col0_in_S_units)
REST = [(0, 1, 0), (1, 2, 0), (2, 3, 0), (3, 2, 1), (4, 3, 1), (5, 3, 2)]


def _plan_sharding(lens):
    order = np.argsort(lens, kind="stable")
    W = []
    for t in range(N_TILES):
        W.append(int(lens[order[t * GROUP:(t + 1) * GROUP]].max()))
    rows = []
    for c in range(N_CORES):
        ids = np.empty(RPC, dtype=np.int64)
        for t in range(N_TILES):
            ids[t * P:(t + 1) * P] = order[
                t * GROUP + c + N_CORES * np.arange(P)]
        rows.append(ids)
    return rows, W


def _plan_layout(lens):
    order = np.argsort(lens, kind="stable")
    slens = lens[order]
    W = [int(slens[t * GROUP:(t + 1) * GROUP].max()) for t in range(N_TILES)]
    S = max(-(-W[t] // (t + 1)) for t in range(N_TILES))
    S = max(512, -(-S // 4) * 4)

    mw = S // 4
    p0s = []
    for k in range(4):
        o = k * mw
        p0 = P
        for t in range(N_TILES):
            gl = slens[t * GROUP:(t + 1) * GROUP]
            cnt = int(np.searchsorted(gl, t * S + o, side="right"))
            p0 = min(p0, cnt // N_CORES)
        p0s.append(p0)
    return dict(S=S, p0s=p0s)


# ---------------------------------------------------------------------------
# Plan tables (iterated against TimelineSim).  Rest-piece coordinates are in
# S/4 units relative to the pred-rest window start (8S); m-chunk coordinates
# are in S/4 units within a diag slot.
#   DMA: ("cmK",) diag chunk K (both halves, all tiles, pruned)
#        (name, "a", o4, w4) align-rest slab   (o4 rel to align window)
#        (name, "p", o4, w4) pred-rest chunk   (o4 rel to pred window)
DMA_CHUNKS = [
    ("ar1", "a", 0, 12), ("cm0",), ("pc4a", "p", 0, 4), ("cm1",),
    ("pc4b", "p", 4, 4), ("cm2",), ("cm3",), ("ar2", "a", 12, 12),
    ("pc5a", "p", 8, 4), ("pc5b", "p", 12, 4), ("pc6", "p", 16, 4),
    ("pc7a", "p", 20, 2), ("pc7b", "p", 22, 1), ("pc8", "p", 23, 1),
]
# Ln: ("lmK",) diag chunk K (align slots), or (name, o4, w4, dep) rest piece
LN_OPS = [
    ("lm0",), ("lm1",), ("lm2",), ("lm3",),
    ("lr0", 0, 4, "ar1"), ("lr1", 4, 4, "ar1"), ("lr2", 8, 4, "ar1"),
    ("lr3", 12, 4, "ar2"), ("lr4", 16, 4, "ar2"), ("lr5", 20, 4, "ar2"),
]
# sub: ("smK",) diag chunk K, or (name, o4, w4, pred chunk, ln dep)
SUB_OPS = [
    ("sm0",), ("sm1",), ("sm2",), ("sm3",),
    ("sc4a", 0, 4, "pc4a", "lr0"), ("sc4b", 4, 4, "pc4b", "lr1"),
    ("sc5a", 8, 4, "pc5a", "lr2"), ("sc5b", 12, 4, "pc5b", "lr3"),
    ("sc6", 16, 4, "pc6", "lr4"), ("sc7a", 20, 2, "pc7a", "lr5"),
    ("sc7b", 22, 1, "pc7b", "lr5"), ("sc8", 23, 1, "pc8", "lr5"),
]
# sq: ("qmKWtT", K, W, T) diag cols [K*q,(K+W)*q) of tile T, or
#     (name, o4, w4, sub dep) rest piece
SQ_OPS = (
    [(f"qm{k}t{t}", k, 1, t) for k in range(4) for t in range(4)]
    + [("qc4a", 0, 4, "sc4a"), ("qc4b", 4, 4, "sc4b"),
       ("qc5a", 8, 4, "sc5a"), ("qc5b", 12, 4, "sc5b"),
       ("qc6", 16, 4, "sc6"), ("qc7a", 20, 2, "sc7a"),
       ("qc7b", 22, 1, "sc7b"), ("qc8", 23, 1, "sc8")]
)
SUB_ENG = {"sc4a": "pool", "sc4b": "pool"}              # default dve
SQ_ENG = {"qc4a": "act", "qc4b": "act", "qc5a": "act", "qc5b": "act",
          "qc6": "act"}                                 # default dve
ACT_ORDER = ["lr0", "lm0", "lr1", "lm1", "lr2", "qc4a", "lm2", "lm3",
             "qc4b", "lr3", "lr4", "lr5", "qc5a", "qc5b", "qc6"]
DVE_ORDER = ["sm0", "qm0t0", "qm0t1", "qm0t2", "qm0t3",
             "sm1", "qm1t0", "qm1t1", "qm1t2", "qm1t3",
             "sm2", "qm2t0", "qm2t1", "qm2t2", "qm2t3",
             "sm3", "qm3t0", "qm3t1", "qm3t2", "qm3t3",
             "sc5a", "sc5b", "sc6", "sc7a", "sc7b", "sc8",
             "qc7a", "qc7b", "qc8"]
POOL_ORDER = ["sc4a", "sc4b"]

MSUB_CHUNK = {f"sm{k}": f"cm{k}" for k in range(4)}
MLN_CHUNK = {f"lm{k}": f"cm{k}" for k in range(4)}


def _build_module(plan):
    S = plan["S"]
    q = S // 4
    PW = 8 * S            # pred-rest window start
    AW = 14 * S           # align-rest window start
    NSQ = len(SQ_OPS)

    nc = bass.Bass("TRN2")
    data_d = nc.dram_tensor("data", [P, 20 * S], F32, kind="ExternalInput")
    out_d = nc.dram_tensor("rowsums", [P, NSQ], F32, kind="ExternalOutput")

    with ExitStack() as ctx:
        db = ctx.enter_context(nc.sbuf_tensor("db", [P, 20 * S], F32))
        sq_sb = ctx.enter_context(nc.sbuf_tensor("sq_sb", [P, 2, S], F32))
        rs_sb = ctx.enter_context(nc.sbuf_tensor("rs_sb", [P, NSQ], F32))
        s_c = {c[0]: ctx.enter_context(nc.semaphore(f"s_{c[0]}"))
               for c in DMA_CHUNKS}
        s_ln = ctx.enter_context(nc.semaphore("s_ln"))
        s_subv = ctx.enter_context(nc.semaphore("s_subv"))
        s_subp = ctx.enter_context(nc.semaphore("s_subp"))
        s_sqa = ctx.enter_context(nc.semaphore("s_sqa"))
        s_sqv = ctx.enter_context(nc.semaphore("s_sqv"))
        s_out = ctx.enter_context(nc.semaphore("s_out"))
        s_ms = ctx.enter_context(nc.semaphore("s_ms"))
        block = ctx.enter_context(nc.Block())

        def slots(x, p0, k, half=None):
            """Diag slot view, chunk k. half: None=all 8, 0=pred, 1=align."""
            v = x[p0:, :8 * S].rearrange("p (s w) -> p s w", s=8)
            v = v[:, :, k * q:(k + 1) * q]
            if half is None:
                return v
            return v[:, half::2, :]

        sub_count = {}
        seq = {"dve": 0, "pool": 0}
        for order in (DVE_ORDER, POOL_ORDER):
            for op in order:
                if op.startswith("s") and not op.startswith("sq"):
                    e = SUB_ENG.get(op, "dve")
                    seq[e] += 1
                    sub_count[op] = (e, seq[e])
        ln_pos = {}
        pos = 0
        for op in ACT_ORDER:
            if op.startswith("l"):
                pos += 1
                ln_pos[op] = pos

        @block.sync
        def _(sync):
            for c in DMA_CHUNKS:
                name = c[0]
                if len(c) == 1:  # diag chunk
                    k = int(name[2])
                    p0 = plan["p0s"][k]
                    sync.dma_start(
                        slots(db, p0, k), slots(data_d, p0, k),
                    ).then_inc(s_c[name], 16)
                else:
                    _n, half, o4, w4 = c
                    base = AW if half == "a" else PW
                    a, b = base + o4 * q, base + (o4 + w4) * q
                    sync.dma_start(
                        db[:, a:b], data_d[:, a:b],
                    ).then_inc(s_c[name], 16)
            n_sqa = sum(1 for s in SQ_OPS if SQ_ENG.get(s[0], "dve") == "act")
            if n_sqa:
                sync.wait_ge(s_sqa, n_sqa)
            if NSQ - n_sqa:
                sync.wait_ge(s_sqv, NSQ - n_sqa)
            sync.dma_start(out_d[:, :], rs_sb[:, :]).then_inc(s_out, 16)

        state = {"nasq": 0}

        def emit_sub(eng_obj, eng_name, op_rec):
            op = op_rec[0]
            if len(op_rec) == 1:  # diag
                k = int(op[2])
                eng_obj.wait_ge(s_c[MSUB_CHUNK[op]], 16)
                eng_obj.wait_ge(s_ln, ln_pos[f"lm{k}"])
                pv, av = slots(db, 0, k, 0), slots(db, 0, k, 1)
            else:
                _n, o4, w4, pch, lndep = op_rec
                eng_obj.wait_ge(s_c[pch], 16)
                eng_obj.wait_ge(s_ln, ln_pos[lndep])
                pv = db[:, PW + o4 * q:PW + (o4 + w4) * q]
                av = db[:, AW + o4 * q:AW + (o4 + w4) * q]
            eng_obj.tensor_sub(pv, pv, av).then_inc(
                s_subv if eng_name == "dve" else s_subp, 1)

        def emit_sq(eng_obj, eng_name, op, emitted_subv):
            rec = next(s for s in SQ_OPS if s[0] == op)
            rs_col = SQ_OPS.index(rec)
            if len(rec) == 4 and isinstance(rec[3], int):  # diag per-tile
                _n, k, kw, t = rec
                subdep = f"sm{k + kw - 1}"
                d = db[:, 2 * t * S + k * q:2 * t * S + (k + kw) * q]
                w = kw * q
            else:
                _n, o4, w4, subdep = rec
                d = db[:, PW + o4 * q:PW + (o4 + w4) * q]
                w = w4 * q
            se, cnt = sub_count[subdep]
            if eng_name != se or (eng_name == "dve" and cnt > emitted_subv):
                eng_obj.wait_ge(s_subv if se == "dve" else s_subp, cnt)
            if eng_name == "act":
                if state["nasq"] >= 2:
                    eng_obj.wait_ge(s_sqa, state["nasq"] - 1)
                eng_obj.activation(
                    sq_sb[:, state["nasq"] % 2, :w], d,
                    mybir.ActivationFunctionType.Square,
                    accum_out=rs_sb[:, rs_col:rs_col + 1],
                ).then_inc(s_sqa, 1)
                state["nasq"] += 1
            else:
                eng_obj.scalar_tensor_tensor(
                    out=d, in0=d, scalar=1.0, in1=d,
                    op0=mybir.AluOpType.mult, op1=mybir.AluOpType.mult,
                    accum_out=rs_sb[:, rs_col:rs_col + 1],
                ).then_inc(s_sqv, 1)

        @block.scalar
        def _(scalar):
            for op in ACT_ORDER:
                if op.startswith("l"):
                    rec = next(l for l in LN_OPS if l[0] == op)
                    if len(rec) == 1:
                        k = int(op[2])
                        scalar.wait_ge(s_c[MLN_CHUNK[op]], 16)
                        ap = slots(db, 0, k, 1)
                    else:
                        _n, o4, w4, dep = rec
                        scalar.wait_ge(s_c[dep], 16)
                        ap = db[:, AW + o4 * q:AW + (o4 + w4) * q]
                    scalar.activation(
                        ap, ap, mybir.ActivationFunctionType.Ln,
                    ).then_inc(s_ln, 1)
                else:
                    emit_sq(scalar, "act", op, 0)

        @block.vector
        def _(v):
            emitted = 0
            for op in DVE_ORDER:
                if op.startswith("sm") or op.startswith("sc"):
                    emit_sub(v, "dve", next(s for s in SUB_OPS if s[0] == op))
                    emitted += 1
                else:
                    emit_sq(v, "dve", op, emitted)

        @block.gpsimd
        def _(g):
            for op in POOL_ORDER:
                emit_sub(g, "pool", next(s for s in SUB_OPS if s[0] == op))

    return nc


def _get_module(plan):
    key = (plan["S"], tuple(plan["p0s"]))
    if key not in _CACHE:
        _CACHE[key] = _build_module(plan)
    return _CACHE[key]


def _pack_core(pred_m, align_m, ids, S):
    buf = np.empty((P, 20 * S), dtype=np.float32)
    buf[:, :8 * S:2 * S] = 0.0  # harmless init; fully overwritten below
    ids_t = ids.reshape(N_TILES, P)
    for t in range(N_TILES):
        col0 = t * S
        w = min(S, T - col0)
        pd = np.zeros((P, S), dtype=np.float32)
        ad = np.ones((P, S), dtype=np.float32)
        if w > 0:
            pd[:, :w] = pred_m[ids_t[t], col0:col0 + w]
            ad[:, :w] = align_m[ids_t[t], col0:col0 + w]
        buf[:, 2 * t * S:(2 * t + 1) * S] = pd
        buf[:, (2 * t + 1) * S:(2 * t + 2) * S] = ad
    for i, t, c0 in REST:
        col0 = c0 * S
        w = min(S, T - col0)
        pr = np.zeros((P, S), dtype=np.float32)
        ar = np.ones((P, S), dtype=np.float32)
        if w > 0:
            pr[:, :w] = pred_m[ids_t[t], col0:col0 + w]
            ar[:, :w] = align_m[ids_t[t], col0:col0 + w]
        buf[:, (8 + i) * S:(9 + i) * S] = pr
        buf[:, (14 + i) * S:(15 + i) * S] = ar
    return buf


def _combine(results, lens, rows, plan):
    total = 0.0
    p0s = plan["p0s"]
    for c in range(N_CORES):
        rs = np.asarray(results[c]["rowsums"], dtype=np.float64)
        rows_sum = np.zeros((P, N_TILES))
        for j, rec in enumerate(SQ_OPS):
            if len(rec) == 4 and isinstance(rec[3], int):
                _n, k, _kw, t = rec
                rows_sum[p0s[k]:, t] += rs[p0s[k]:, j]
            else:
                _n, o4, w4, _sub = rec
                a = o4
                while w4 > 0:
                    ri = a // 4
                    rows_sum[:, REST[ri][1]] += rs[:, j]
                    break  # pieces never cross region boundaries
        per_row = rows_sum.T.reshape(RPC)
        lc = lens[rows[c]].astype(np.float64)
        total += np.sum(per_row / lc)
    return np.array(total / B, dtype=np.float32)


def run(inputs, trace: bool = False):
    pred = np.asarray(inputs["pred"], dtype=np.float32)
    align = np.asarray(inputs["alignment"], dtype=np.float32)
    lens = np.asarray(inputs["token_lengths"])

    rows, W = _plan_sharding(lens)
    plan = _plan_layout(lens)
    nc = _get_module(plan)

    col = np.arange(T)[None, :]
    lcol = lens[:, None]
    pred_m = np.where(col < lcol, pred, 0.0).astype(np.float32)
    align_m = np.where(col < lcol, align, 1.0).astype(np.float32)

    in_maps = [{"data": _pack_core(pred_m, align_m, rows[c], plan["S"])}
               for c in range(N_CORES)]

    res = run_bass_kernel_spmd(nc, in_maps, core_ids=list(range(N_CORES)),
                               trace=trace)
    return _combine(res.results, lens, rows, plan), res


def kernel(**inputs) -> np.ndarray:
    out, _ = run(inputs, trace=False)
    return out


def _sim_module(lens):
    return _get_module(_plan_layout(np.asarray(lens)))


# revision 28
# speedup vs baseline: 1.0064x; 1.0064x over previous
"""Masked per-sample MSE loss (duration-predictor loss) on 8 Trainium2 cores.

Math (per the reference):
    mask[i, j]  = j < token_lengths[i]
    diff        = where(mask, pred - log(alignment), 0.0)
    out         = mean_i( sum_j diff[i,j]^2 / token_lengths[i] )

Sharding: data parallel over the batch dim with length-sorted row assignment.
Rows sorted by token_length; sorted rank r -> row-tile t = r // 1024, core
c = r % 8, partition p = (r % 1024) // 8. Tile t's rows all have length <=
W[t]; with S = max_t ceil(W[t]/(t+1)), tile t is treated as covering columns
[0, S*(t+1)).

Host-side packing: each core's pred and align rows are packed into ONE
[128, 20S] f32 DRAM buffer:
  [0,8S):    diag slots  [pd0 ad0 pd1 ad1 pd2 ad2 pd3 ad3], each S wide,
             where pd_t/ad_t = pred/align of tile t, cols [S*t, S*(t+1))
  [8S,14S):  pred rest   [t1[0,S) t2[0,S) t3[0,S) t2[S,2S) t3[S,2S) t3[2S,3S)]
  [14S,20S): align rest  (same region order)
Invalid positions are padded pred=0 / align=1 (ln 1 = 0), so the device
needs NO masks, iota, or length tensors.  The 8 diag slots have uniform
stride S, so ONE 3-dim DMA [p, slot, w] fetches the same column window of
both halves of all four tiles; sorted order lets diag chunks drop an exact
prefix of partitions — pure traffic pruning to ~52% of the dense bytes.
HWDGE descriptor generation is 625 ns serial per DMA, so DMA count is the
scarce resource: 14 loads cover everything, aligns in two big early slabs
and pred-rest in fine chunks so the tail chain stays short.

Device work: Ln(align) in place (ACT), d = pred - ln(align) in place (DVE or
Pool tensor_sub), then per-(piece, tile) square-with-row-sum-accum (DVE
scalar_tensor_tensor in place, ACT Square into scratch for a few big slices)
into a distinct rs column.  Compute always runs on all 128 partitions (the
ISA forbids partition-offset compute); rows below a pruned chunk's p0 leave
garbage in that chunk's rs column, which the host combine ignores.

Per-row divide by length and the global mean run on the host in float64.

Raw Bass with explicit semaphores (the walrus build rejects compute
instructions carrying more than one sync-wait, so waits are standalone)."""

from contextlib import ExitStack

import numpy as np

import concourse.bass as bass
from concourse import mybir
from concourse.bass_utils import run_bass_kernel_spmd

B, T = 4096, 2048
N_CORES = 8
RPC = B // N_CORES
P = 128
N_TILES = RPC // P
GROUP = P * N_CORES

F32 = mybir.dt.float32

_CACHE: dict = {}

# rest regions: (index, tile, col0_in_S_units)
REST = [(0, 1, 0), (1, 2, 0), (2, 3, 0), (3, 2, 1), (4, 3, 1), (5, 3, 2)]


def _plan_sharding(lens):
    order = np.argsort(lens, kind="stable")
    W = []
    for t in range(N_TILES):
        W.append(int(lens[order[t * GROUP:(t + 1) * GROUP]].max()))
    rows = []
    for c in range(N_CORES):
        ids = np.empty(RPC, dtype=np.int64)
        for t in range(N_TILES):
            ids[t * P:(t + 1) * P] = order[
                t * GROUP + c + N_CORES * np.arange(P)]
        rows.append(ids)
    return rows, W


def _plan_layout(lens):
    order = np.argsort(lens, kind="stable")
    slens = lens[order]
    W = [int(slens[t * GROUP:(t + 1) * GROUP].max()) for t in range(N_TILES)]
    S = max(-(-W[t] // (t + 1)) for t in range(N_TILES))
    S = max(512, -(-S // 4) * 4)

    mw = S // 4
    p0s = []
    for k in range(4):
        o = k * mw
        p0 = P
        for t in range(N_TILES):
            gl = slens[t * GROUP:(t + 1) * GROUP]
            cnt = int(np.searchsorted(gl, t * S + o, side="right"))
            p0 = min(p0, cnt // N_CORES)
        p0s.append(p0)
    return dict(S=S, p0s=p0s)


# ---------------------------------------------------------------------------
# Plan tables (iterated against TimelineSim).  Rest-piece coordinates are in
# S/4 units relative to the pred-rest window start (8S); m-chunk coordinates
# are in S/4 units within a diag slot.
#   DMA: ("cmK",) diag chunk K (both halves, all tiles, pruned)
#        (name, "a", o4, w4) align-rest slab   (o4 rel to align window)
#        (name, "p", o4, w4) pred-rest chunk   (o4 rel to pred window)
DMA_CHUNKS = [
    ("ar1", "a", 0, 12), ("cm0",), ("pc4a", "p", 0, 4), ("cm1",),
    ("pc4b", "p", 4, 4), ("cm2",), ("cm3",), ("ar2", "a", 12, 12),
    ("pc5a", "p", 8, 4), ("pc5b", "p", 12, 4), ("pc6", "p", 16, 4),
    ("pc7a", "p", 20, 2), ("pc7b", "p", 22, 1), ("pc8", "p", 23, 1),
]
# Ln: ("lmK",) diag chunk K (align slots), or (name, o4, w4, dep) rest piece
LN_OPS = [
    ("lm0",), ("lm1",), ("lm2",), ("lm3",),
    ("lr0", 0, 4, "ar1"), ("lr1", 4, 4, "ar1"), ("lr2", 8, 4, "ar1"),
    ("lr3", 12, 4, "ar2"), ("lr4", 16, 4, "ar2"), ("lr5", 20, 4, "ar2"),
]
# sub: ("smK",) diag chunk K, or (name, o4, w4, pred chunk, ln dep)
SUB_OPS = [
    ("sm0",), ("sm1",), ("sm2",), ("sm3",),
    ("sc4a", 0, 4, "pc4a", "lr0"), ("sc4b", 4, 4, "pc4b", "lr1"),
    ("sc5a", 8, 4, "pc5a", "lr2"), ("sc5b", 12, 4, "pc5b", "lr3"),
    ("sc6", 16, 4, "pc6", "lr4"), ("sc7a", 20, 2, "pc7a", "lr5"),
    ("sc7b", 22, 1, "pc7b", "lr5"), ("sc8", 23, 1, "pc8", "lr5"),
]
# sq: ("qmKWtT", K, W, T) diag cols [K*q,(K+W)*q) of tile T, or
#     (name, o4, w4, sub dep) rest piece
SQ_OPS = (
    [(f"qm{k}t{t}", k, 1, t) for k in range(4) for t in range(4)]
    + [("qc4a", 0, 4, "sc4a"), ("qc4b", 4, 4, "sc4b"),
       ("qc5a", 8, 4, "sc5a"), ("qc5b", 12, 4, "sc5b"),
       ("qc6", 16, 4, "sc6"), ("qc7a", 20, 2, "sc7a"),
       ("qc7b", 22, 1, "sc7b"), ("qc8", 23, 1, "sc8")]
)
SUB_ENG = {"sc4a": "pool", "sc4b": "pool"}              # default dve
SQ_ENG = {"qc4a": "act", "qc4b": "act", "qc5a": "act", "qc5b": "act",
          "qc6": "act"}                                 # default dve
ACT_ORDER = ["lr0", "lr1", "lm0", "lr2", "lm1", "qc4a", "lm2", "lm3",
             "qc4b", "lr3", "lr4", "lr5", "qc5a", "qc5b", "qc6"]
DVE_ORDER = ["sm0", "qm0t0", "qm0t1", "qm0t2", "qm0t3",
             "sm1", "qm1t0", "qm1t1", "sm2", "qm1t2", "qm1t3",
             "qm2t0", "qm2t1", "sm3", "qm2t2", "qm2t3",
             "qm3t0", "qm3t1", "qm3t2", "qm3t3",
             "sc5a", "sc5b", "sc6", "sc7a", "sc7b", "sc8",
             "qc7a", "qc7b", "qc8"]
POOL_ORDER = ["sc4a", "sc4b"]

MSUB_CHUNK = {f"sm{k}": f"cm{k}" for k in range(4)}
MLN_CHUNK = {f"lm{k}": f"cm{k}" for k in range(4)}


def _build_module(plan):
    S = plan["S"]
    q = S // 4
    PW = 8 * S            # pred-rest window start
    AW = 14 * S           # align-rest window start
    NSQ = len(SQ_OPS)

    nc = bass.Bass("TRN2")
    data_d = nc.dram_tensor("data", [P, 20 * S], F32, kind="ExternalInput")
    out_d = nc.dram_tensor("rowsums", [P, NSQ], F32, kind="ExternalOutput")

    with ExitStack() as ctx:
        db = ctx.enter_context(nc.sbuf_tensor("db", [P, 20 * S], F32))
        sq_sb = ctx.enter_context(nc.sbuf_tensor("sq_sb", [P, 2, S], F32))
        rs_sb = ctx.enter_context(nc.sbuf_tensor("rs_sb", [P, NSQ], F32))
        s_c = {c[0]: ctx.enter_context(nc.semaphore(f"s_{c[0]}"))
               for c in DMA_CHUNKS}
        s_ln = ctx.enter_context(nc.semaphore("s_ln"))
        s_subv = ctx.enter_context(nc.semaphore("s_subv"))
        s_subp = ctx.enter_context(nc.semaphore("s_subp"))
        s_sqa = ctx.enter_context(nc.semaphore("s_sqa"))
        s_sqv = ctx.enter_context(nc.semaphore("s_sqv"))
        s_out = ctx.enter_context(nc.semaphore("s_out"))
        s_ms = ctx.enter_context(nc.semaphore("s_ms"))
        block = ctx.enter_context(nc.Block())

        def slots(x, p0, k, half=None):
            """Diag slot view, chunk k. half: None=all 8, 0=pred, 1=align."""
            v = x[p0:, :8 * S].rearrange("p (s w) -> p s w", s=8)
            v = v[:, :, k * q:(k + 1) * q]
            if half is None:
                return v
            return v[:, half::2, :]

        sub_count = {}
        seq = {"dve": 0, "pool": 0}
        for order in (DVE_ORDER, POOL_ORDER):
            for op in order:
                if op.startswith("s") and not op.startswith("sq"):
                    e = SUB_ENG.get(op, "dve")
                    seq[e] += 1
                    sub_count[op] = (e, seq[e])
        ln_pos = {}
        pos = 0
        for op in ACT_ORDER:
            if op.startswith("l"):
                pos += 1
                ln_pos[op] = pos

        @block.sync
        def _(sync):
            for c in DMA_CHUNKS:
                name = c[0]
                if len(c) == 1:  # diag chunk
                    k = int(name[2])
                    p0 = plan["p0s"][k]
                    sync.dma_start(
                        slots(db, p0, k), slots(data_d, p0, k),
                    ).then_inc(s_c[name], 16)
                else:
                    _n, half, o4, w4 = c
                    base = AW if half == "a" else PW
                    a, b = base + o4 * q, base + (o4 + w4) * q
                    sync.dma_start(
                        db[:, a:b], data_d[:, a:b],
                    ).then_inc(s_c[name], 16)
            n_sqa = sum(1 for s in SQ_OPS if SQ_ENG.get(s[0], "dve") == "act")
            if n_sqa:
                sync.wait_ge(s_sqa, n_sqa)
            if NSQ - n_sqa:
                sync.wait_ge(s_sqv, NSQ - n_sqa)
            sync.dma_start(out_d[:, :], rs_sb[:, :]).then_inc(s_out, 16)

        state = {"nasq": 0}

        def emit_sub(eng_obj, eng_name, op_rec):
            op = op_rec[0]
            if len(op_rec) == 1:  # diag
                k = int(op[2])
                eng_obj.wait_ge(s_c[MSUB_CHUNK[op]], 16)
                eng_obj.wait_ge(s_ln, ln_pos[f"lm{k}"])
                pv, av = slots(db, 0, k, 0), slots(db, 0, k, 1)
            else:
                _n, o4, w4, pch, lndep = op_rec
                eng_obj.wait_ge(s_c[pch], 16)
                eng_obj.wait_ge(s_ln, ln_pos[lndep])
                pv = db[:, PW + o4 * q:PW + (o4 + w4) * q]
                av = db[:, AW + o4 * q:AW + (o4 + w4) * q]
            eng_obj.tensor_sub(pv, pv, av).then_inc(
                s_subv if eng_name == "dve" else s_subp, 1)

        def emit_sq(eng_obj, eng_name, op, emitted_subv):
            rec = next(s for s in SQ_OPS if s[0] == op)
            rs_col = SQ_OPS.index(rec)
            if len(rec) == 4 and isinstance(rec[3], int):  # diag per-tile
                _n, k, kw, t = rec
                subdep = f"sm{k + kw - 1}"
                d = db[:, 2 * t * S + k * q:2 * t * S + (k + kw) * q]
                w = kw * q
            else:
                _n, o4, w4, subdep = rec
                d = db[:, PW + o4 * q:PW + (o4 + w4) * q]
                w = w4 * q
            se, cnt = sub_count[subdep]
            if eng_name != se or (eng_name == "dve" and cnt > emitted_subv):
                eng_obj.wait_ge(s_subv if se == "dve" else s_subp, cnt)
            if eng_name == "act":
                if state["nasq"] >= 2:
                    eng_obj.wait_ge(s_sqa, state["nasq"] - 1)
                eng_obj.activation(
                    sq_sb[:, state["nasq"] % 2, :w], d,
                    mybir.ActivationFunctionType.Square,
                    accum_out=rs_sb[:, rs_col:rs_col + 1],
                ).then_inc(s_sqa, 1)
                state["nasq"] += 1
            else:
                eng_obj.scalar_tensor_tensor(
                    out=d, in0=d, scalar=1.0, in1=d,
                    op0=mybir.AluOpType.mult, op1=mybir.AluOpType.mult,
                    accum_out=rs_sb[:, rs_col:rs_col + 1],
                ).then_inc(s_sqv, 1)

        @block.scalar
        def _(scalar):
            for op in ACT_ORDER:
                if op.startswith("l"):
                    rec = next(l for l in LN_OPS if l[0] == op)
                    if len(rec) == 1:
                        k = int(op[2])
                        scalar.wait_ge(s_c[MLN_CHUNK[op]], 16)
                        ap = slots(db, 0, k, 1)
                    else:
                        _n, o4, w4, dep = rec
                        scalar.wait_ge(s_c[dep], 16)
                        ap = db[:, AW + o4 * q:AW + (o4 + w4) * q]
                    scalar.activation(
                        ap, ap, mybir.ActivationFunctionType.Ln,
                    ).then_inc(s_ln, 1)
                else:
                    emit_sq(scalar, "act", op, 0)

        @block.vector
        def _(v):
            emitted = 0
            for op in DVE_ORDER:
                if op.startswith("sm") or op.startswith("sc"):
                    emit_sub(v, "dve", next(s for s in SUB_OPS if s[0] == op))
                    emitted += 1
                else:
                    emit_sq(v, "dve", op, emitted)

        @block.gpsimd
        def _(g):
            for op in POOL_ORDER:
                emit_sub(g, "pool", next(s for s in SUB_OPS if s[0] == op))

    return nc


def _get_module(plan):
    key = (plan["S"], tuple(plan["p0s"]))
    if key not in _CACHE:
        _CACHE[key] = _build_module(plan)
    return _CACHE[key]


def _pack_core(pred_m, align_m, ids, S):
    buf = np.empty((P, 20 * S), dtype=np.float32)
    buf[:, :8 * S:2 * S] = 0.0  # harmless init; fully overwritten below
    ids_t = ids.reshape(N_TILES, P)
    for t in range(N_TILES):
        col0 = t * S
        w = min(S, T - col0)
        pd = np.zeros((P, S), dtype=np.float32)
        ad = np.ones((P, S), dtype=np.float32)
        if w > 0:
            pd[:, :w] = pred_m[ids_t[t], col0:col0 + w]
            ad[:, :w] = align_m[ids_t[t], col0:col0 + w]
        buf[:, 2 * t * S:(2 * t + 1) * S] = pd
        buf[:, (2 * t + 1) * S:(2 * t + 2) * S] = ad
    for i, t, c0 in REST:
        col0 = c0 * S
        w = min(S, T - col0)
        pr = np.zeros((P, S), dtype=np.float32)
        ar = np.ones((P, S), dtype=np.float32)
        if w > 0:
            pr[:, :w] = pred_m[ids_t[t], col0:col0 + w]
            ar[:, :w] = align_m[ids_t[t], col0:col0 + w]
        buf[:, (8 + i) * S:(9 + i) * S] = pr
        buf[:, (14 + i) * S:(15 + i) * S] = ar
    return buf


def _combine(results, lens, rows, plan):
    total = 0.0
    p0s = plan["p0s"]
    for c in range(N_CORES):
        rs = np.asarray(results[c]["rowsums"], dtype=np.float64)
        rows_sum = np.zeros((P, N_TILES))
        for j, rec in enumerate(SQ_OPS):
            if len(rec) == 4 and isinstance(rec[3], int):
                _n, k, _kw, t = rec
                rows_sum[p0s[k]:, t] += rs[p0s[k]:, j]
            else:
                _n, o4, w4, _sub = rec
                a = o4
                while w4 > 0:
                    ri = a // 4
                    rows_sum[:, REST[ri][1]] += rs[:, j]
                    break  # pieces never cross region boundaries
        per_row = rows_sum.T.reshape(RPC)
        lc = lens[rows[c]].astype(np.float64)
        total += np.sum(per_row / lc)
    return np.array(total / B, dtype=np.float32)


def run(inputs, trace: bool = False):
    pred = np.asarray(inputs["pred"], dtype=np.float32)
    align = np.asarray(inputs["alignment"], dtype=np.float32)
    lens = np.asarray(inputs["token_lengths"])

    rows, W = _plan_sharding(lens)
    plan = _plan_layout(lens)
    nc = _get_module(plan)

    col = np.arange(T)[None, :]
    lcol = lens[:, None]
    pred_m = np.where(col < lcol, pred, 0.0).astype(np.float32)
    align_m = np.where(col < lcol, align, 1.0).astype(np.float32)

    in_maps = [{"data": _pack_core(pred_m, align_m, rows[c], plan["S"])}
               for c in range(N_CORES)]

    res = run_bass_kernel_spmd(nc, in_maps, core_ids=list(range(N_CORES)),
                               trace=trace)
    return _combine(res.results, lens, rows, plan), res


def kernel(**inputs) -> np.ndarray:
    out, _ = run(inputs, trace=False)
    return out


def _sim_module(lens):
    return _get_module(_plan_layout(np.asarray(lens)))


# revision 29
# speedup vs baseline: 1.0263x; 1.0198x over previous
"""Masked per-sample MSE loss (duration-predictor loss) on 8 Trainium2 cores.

Math (per the reference):
    mask[i, j]  = j < token_lengths[i]
    diff        = where(mask, pred - log(alignment), 0.0)
    out         = mean_i( sum_j diff[i,j]^2 / token_lengths[i] )

Sharding: data parallel over the batch dim with length-sorted row assignment.
Rows sorted by token_length; sorted rank r -> row-tile t = r // 1024, core
c = r % 8, partition p = (r % 1024) // 8. Tile t's rows all have length <=
W[t]; with S = max_t ceil(W[t]/(t+1)), tile t is treated as covering columns
[0, S*(t+1)).

Host-side packing: each core's pred and align rows are packed into ONE
[128, 20S] f32 DRAM buffer:
  [0,8S):    diag slots  [pd0 ad0 pd1 ad1 pd2 ad2 pd3 ad3], each S wide,
             where pd_t/ad_t = pred/align of tile t, cols [S*t, S*(t+1))
  [8S,14S):  pred rest   [t1[0,S) t2[0,S) t3[0,S) t2[S,2S) t3[S,2S) t3[2S,3S)]
  [14S,20S): align rest  (same region order)
Invalid positions are padded pred=0 / align=1 (ln 1 = 0), so the device
needs NO masks, iota, or length tensors.  The 8 diag slots have uniform
stride S, so ONE 3-dim DMA [p, slot, w] fetches the same column window of
both halves of all four tiles; sorted order lets diag chunks drop an exact
prefix of partitions — pure traffic pruning to ~52% of the dense bytes.
HWDGE descriptor generation is 625 ns serial per DMA, so DMA count is the
scarce resource: 14 loads cover everything, aligns in two big early slabs
and pred-rest in fine chunks so the tail chain stays short.

Device work: Ln(align) in place (ACT), d = pred - ln(align) in place (DVE or
Pool tensor_sub), then per-(piece, tile) square-with-row-sum-accum (DVE
scalar_tensor_tensor in place, ACT Square into scratch for a few big slices)
into a distinct rs column.  Compute always runs on all 128 partitions (the
ISA forbids partition-offset compute); rows below a pruned chunk's p0 leave
garbage in that chunk's rs column, which the host combine ignores.

Per-row divide by length and the global mean run on the host in float64.

Raw Bass with explicit semaphores (the walrus build rejects compute
instructions carrying more than one sync-wait, so waits are standalone)."""

from contextlib import ExitStack

import numpy as np

import concourse.bass as bass
from concourse import mybir
from concourse.bass_utils import run_bass_kernel_spmd

B, T = 4096, 2048
N_CORES = 8
RPC = B // N_CORES
P = 128
N_TILES = RPC // P
GROUP = P * N_CORES

F32 = mybir.dt.float32

_CACHE: dict = {}

# rest regions: (index, tile, col0_in_S_units)
REST = [(0, 1, 0), (1, 2, 0), (2, 3, 0), (3, 2, 1), (4, 3, 1), (5, 3, 2)]


def _plan_sharding(lens):
    order = np.argsort(lens, kind="stable")
    W = []
    for t in range(N_TILES):
        W.append(int(lens[order[t * GROUP:(t + 1) * GROUP]].max()))
    rows = []
    for c in range(N_CORES):
        ids = np.empty(RPC, dtype=np.int64)
        for t in range(N_TILES):
            ids[t * P:(t + 1) * P] = order[
                t * GROUP + c + N_CORES * np.arange(P)]
        rows.append(ids)
    return rows, W


def _plan_layout(lens):
    order = np.argsort(lens, kind="stable")
    slens = lens[order]
    W = [int(slens[t * GROUP:(t + 1) * GROUP].max()) for t in range(N_TILES)]
    S = max(-(-W[t] // (t + 1)) for t in range(N_TILES))
    S = max(512, -(-S // 4) * 4)

    mw = S // 4
    p0s = []
    for k in range(4):
        o = k * mw
        p0 = P
        for t in range(N_TILES):
            gl = slens[t * GROUP:(t + 1) * GROUP]
            cnt = int(np.searchsorted(gl, t * S + o, side="right"))
            p0 = min(p0, cnt // N_CORES)
        p0s.append(p0)
    return dict(S=S, p0s=p0s)


# ---------------------------------------------------------------------------
# Plan tables (iterated against TimelineSim).  Rest-piece coordinates are in
# S/4 units relative to the pred-rest window start (8S); m-chunk coordinates
# are in S/4 units within a diag slot.
#   DMA: ("cmK",) diag chunk K (both halves, all tiles, pruned)
#        (name, "a", o4, w4) align-rest slab   (o4 rel to align window)
#        (name, "p", o4, w4) pred-rest chunk   (o4 rel to pred window)
DMA_CHUNKS = [
    ("ar1", "a", 0, 12), ("cm0",), ("pc4a", "p", 0, 4), ("cm1",),
    ("pc4b", "p", 4, 4), ("cm2",), ("cm3",), ("ar2", "a", 12, 12),
    ("pc5a", "p", 8, 4), ("pc5b", "p", 12, 4), ("pc6", "p", 16, 4),
    ("pc7a", "p", 20, 2), ("pc7b", "p", 22, 1), ("pc8", "p", 23, 1),
]
# Ln: ("lmK",) diag chunk K (align slots), or (name, o4, w4, dep) rest piece
LN_OPS = [
    ("lm0",), ("lm1",), ("lm2",), ("lm3",),
    ("lr0", 0, 4, "ar1"), ("lr1", 4, 4, "ar1"), ("lr2", 8, 4, "ar1"),
    ("lr3", 12, 4, "ar2"), ("lr4", 16, 4, "ar2"), ("lr5", 20, 4, "ar2"),
]
# sub: ("smK",) diag chunk K, or (name, o4, w4, pred chunk, ln dep)
SUB_OPS = [
    ("sm0",), ("sm1",), ("sm2",), ("sm3",),
    ("sc4a", 0, 4, "pc4a", "lr0"), ("sc4b", 4, 4, "pc4b", "lr1"),
    ("sc5a", 8, 4, "pc5a", "lr2"), ("sc5b", 12, 4, "pc5b", "lr3"),
    ("sc6", 16, 4, "pc6", "lr4"), ("sc7a", 20, 2, "pc7a", "lr5"),
    ("sc7b", 22, 1, "pc7b", "lr5"), ("sc8", 23, 1, "pc8", "lr5"),
]
# sq: ("qmKWtT", K, W, T) diag cols [K*q,(K+W)*q) of tile T, or
#     (name, o4, w4, sub dep) rest piece
SQ_OPS = (
    [(f"qm{k}t{t}", k, 1, t) for k in range(4) for t in range(4)]
    + [("qc4a", 0, 4, "sc4a"), ("qc4b", 4, 4, "sc4b"),
       ("qc5a", 8, 4, "sc5a"), ("qc5b", 12, 4, "sc5b"),
       ("qc6", 16, 4, "sc6"), ("qc7a", 20, 2, "sc7a"),
       ("qc7b", 22, 1, "sc7b"), ("qc8", 23, 1, "sc8")]
)
SUB_ENG = {"sc4a": "pool", "sc4b": "pool"}              # default dve
SQ_ENG = {"qc4a": "act", "qc4b": "act", "qc5a": "act", "qc5b": "act",
          "qc6": "act"}                                 # default dve
ACT_ORDER = ["lr0", "lr1", "lm0", "lr2", "lm1", "qc4a", "lm2", "lm3",
             "qc4b", "lr3", "lr4", "lr5", "qc5a", "qc5b", "qc6"]
DVE_ORDER = ["sm0", "qm0t0", "qm0t1", "qm0t2", "qm0t3",
             "sm1", "qm1t0", "qm1t1", "qm1t2", "qm1t3",
             "sm2", "qm2t0", "qm2t1", "qm2t2", "qm2t3",
             "sm3", "qm3t0", "qm3t1", "qm3t2", "qm3t3",
             "sc5a", "sc5b", "sc6", "sc7a", "sc7b", "sc8",
             "qc7a", "qc7b", "qc8"]
POOL_ORDER = ["sc4a", "sc4b"]

MSUB_CHUNK = {f"sm{k}": f"cm{k}" for k in range(4)}
MLN_CHUNK = {f"lm{k}": f"cm{k}" for k in range(4)}


def _build_module(plan):
    S = plan["S"]
    q = S // 4
    PW = 8 * S            # pred-rest window start
    AW = 14 * S           # align-rest window start
    NSQ = len(SQ_OPS)

    nc = bass.Bass("TRN2")
    data_d = nc.dram_tensor("data", [P, 20 * S], F32, kind="ExternalInput")
    out_d = nc.dram_tensor("rowsums", [P, NSQ], F32, kind="ExternalOutput")

    with ExitStack() as ctx:
        db = ctx.enter_context(nc.sbuf_tensor("db", [P, 20 * S], F32))
        sq_sb = ctx.enter_context(nc.sbuf_tensor("sq_sb", [P, 2, S], F32))
        rs_sb = ctx.enter_context(nc.sbuf_tensor("rs_sb", [P, NSQ], F32))
        s_c = {c[0]: ctx.enter_context(nc.semaphore(f"s_{c[0]}"))
               for c in DMA_CHUNKS}
        s_ln = ctx.enter_context(nc.semaphore("s_ln"))
        s_subv = ctx.enter_context(nc.semaphore("s_subv"))
        s_subp = ctx.enter_context(nc.semaphore("s_subp"))
        s_sqa = ctx.enter_context(nc.semaphore("s_sqa"))
        s_sqv = ctx.enter_context(nc.semaphore("s_sqv"))
        s_out = ctx.enter_context(nc.semaphore("s_out"))
        s_ms = ctx.enter_context(nc.semaphore("s_ms"))
        block = ctx.enter_context(nc.Block())

        def slots(x, p0, k, half=None):
            """Diag slot view, chunk k. half: None=all 8, 0=pred, 1=align."""
            v = x[p0:, :8 * S].rearrange("p (s w) -> p s w", s=8)
            v = v[:, :, k * q:(k + 1) * q]
            if half is None:
                return v
            return v[:, half::2, :]

        sub_count = {}
        seq = {"dve": 0, "pool": 0}
        for order in (DVE_ORDER, POOL_ORDER):
            for op in order:
                if op.startswith("s") and not op.startswith("sq"):
                    e = SUB_ENG.get(op, "dve")
                    seq[e] += 1
                    sub_count[op] = (e, seq[e])
        ln_pos = {}
        pos = 0
        for op in ACT_ORDER:
            if op.startswith("l"):
                pos += 1
                ln_pos[op] = pos

        @block.sync
        def _(sync):
            for c in DMA_CHUNKS:
                name = c[0]
                if len(c) == 1:  # diag chunk
                    k = int(name[2])
                    p0 = plan["p0s"][k]
                    sync.dma_start(
                        slots(db, p0, k), slots(data_d, p0, k),
                    ).then_inc(s_c[name], 16)
                else:
                    _n, half, o4, w4 = c
                    base = AW if half == "a" else PW
                    a, b = base + o4 * q, base + (o4 + w4) * q
                    sync.dma_start(
                        db[:, a:b], data_d[:, a:b],
                    ).then_inc(s_c[name], 16)
            n_sqa = sum(1 for s in SQ_OPS if SQ_ENG.get(s[0], "dve") == "act")
            if n_sqa:
                sync.wait_ge(s_sqa, n_sqa)
            if NSQ - n_sqa:
                sync.wait_ge(s_sqv, NSQ - n_sqa)
            sync.dma_start(out_d[:, :], rs_sb[:, :]).then_inc(s_out, 16)

        state = {"nasq": 0}

        def emit_sub(eng_obj, eng_name, op_rec):
            op = op_rec[0]
            if len(op_rec) == 1:  # diag
                k = int(op[2])
                eng_obj.wait_ge(s_c[MSUB_CHUNK[op]], 16)
                eng_obj.wait_ge(s_ln, ln_pos[f"lm{k}"])
                pv, av = slots(db, 0, k, 0), slots(db, 0, k, 1)
            else:
                _n, o4, w4, pch, lndep = op_rec
                eng_obj.wait_ge(s_c[pch], 16)
                eng_obj.wait_ge(s_ln, ln_pos[lndep])
                pv = db[:, PW + o4 * q:PW + (o4 + w4) * q]
                av = db[:, AW + o4 * q:AW + (o4 + w4) * q]
            eng_obj.tensor_sub(pv, pv, av).then_inc(
                s_subv if eng_name == "dve" else s_subp, 1)

        def emit_sq(eng_obj, eng_name, op, emitted_subv):
            rec = next(s for s in SQ_OPS if s[0] == op)
            rs_col = SQ_OPS.index(rec)
            if len(rec) == 4 and isinstance(rec[3], int):  # diag per-tile
                _n, k, kw, t = rec
                subdep = f"sm{k + kw - 1}"
                d = db[:, 2 * t * S + k * q:2 * t * S + (k + kw) * q]
                w = kw * q
            else:
                _n, o4, w4, subdep = rec
                d = db[:, PW + o4 * q:PW + (o4 + w4) * q]
                w = w4 * q
            se, cnt = sub_count[subdep]
            if eng_name != se or (eng_name == "dve" and cnt > emitted_subv):
                eng_obj.wait_ge(s_subv if se == "dve" else s_subp, cnt)
            if eng_name == "act":
                if state["nasq"] >= 2:
                    eng_obj.wait_ge(s_sqa, state["nasq"] - 1)
                eng_obj.activation(
                    sq_sb[:, state["nasq"] % 2, :w], d,
                    mybir.ActivationFunctionType.Square,
                    accum_out=rs_sb[:, rs_col:rs_col + 1],
                ).then_inc(s_sqa, 1)
                state["nasq"] += 1
            else:
                eng_obj.scalar_tensor_tensor(
                    out=d, in0=d, scalar=1.0, in1=d,
                    op0=mybir.AluOpType.mult, op1=mybir.AluOpType.mult,
                    accum_out=rs_sb[:, rs_col:rs_col + 1],
                ).then_inc(s_sqv, 1)

        @block.scalar
        def _(scalar):
            for op in ACT_ORDER:
                if op.startswith("l"):
                    rec = next(l for l in LN_OPS if l[0] == op)
                    if len(rec) == 1:
                        k = int(op[2])
                        scalar.wait_ge(s_c[MLN_CHUNK[op]], 16)
                        ap = slots(db, 0, k, 1)
                    else:
                        _n, o4, w4, dep = rec
                        scalar.wait_ge(s_c[dep], 16)
                        ap = db[:, AW + o4 * q:AW + (o4 + w4) * q]
                    scalar.activation(
                        ap, ap, mybir.ActivationFunctionType.Ln,
                    ).then_inc(s_ln, 1)
                else:
                    emit_sq(scalar, "act", op, 0)

        @block.vector
        def _(v):
            emitted = 0
            for op in DVE_ORDER:
                if op.startswith("sm") or op.startswith("sc"):
                    emit_sub(v, "dve", next(s for s in SUB_OPS if s[0] == op))
                    emitted += 1
                else:
                    emit_sq(v, "dve", op, emitted)

        @block.gpsimd
        def _(g):
            for op in POOL_ORDER:
                emit_sub(g, "pool", next(s for s in SUB_OPS if s[0] == op))

    return nc


def _get_module(plan):
    key = (plan["S"], tuple(plan["p0s"]))
    if key not in _CACHE:
        _CACHE[key] = _build_module(plan)
    return _CACHE[key]


def _pack_core(pred_m, align_m, ids, S):
    buf = np.empty((P, 20 * S), dtype=np.float32)
    buf[:, :8 * S:2 * S] = 0.0  # harmless init; fully overwritten below
    ids_t = ids.reshape(N_TILES, P)
    for t in range(N_TILES):
        col0 = t * S
        w = min(S, T - col0)
        pd = np.zeros((P, S), dtype=np.float32)
        ad = np.ones((P, S), dtype=np.float32)
        if w > 0:
            pd[:, :w] = pred_m[ids_t[t], col0:col0 + w]
            ad[:, :w] = align_m[ids_t[t], col0:col0 + w]
        buf[:, 2 * t * S:(2 * t + 1) * S] = pd
        buf[:, (2 * t + 1) * S:(2 * t + 2) * S] = ad
    for i, t, c0 in REST:
        col0 = c0 * S
        w = min(S, T - col0)
        pr = np.zeros((P, S), dtype=np.float32)
        ar = np.ones((P, S), dtype=np.float32)
        if w > 0:
            pr[:, :w] = pred_m[ids_t[t], col0:col0 + w]
            ar[:, :w] = align_m[ids_t[t], col0:col0 + w]
        buf[:, (8 + i) * S:(9 + i) * S] = pr
        buf[:, (14 + i) * S:(15 + i) * S] = ar
    return buf


def _combine(results, lens, rows, plan):
    total = 0.0
    p0s = plan["p0s"]
    for c in range(N_CORES):
        rs = np.asarray(results[c]["rowsums"], dtype=np.float64)
        rows_sum = np.zeros((P, N_TILES))
        for j, rec in enumerate(SQ_OPS):
            if len(rec) == 4 and isinstance(rec[3], int):
                _n, k, _kw, t = rec
                rows_sum[p0s[k]:, t] += rs[p0s[k]:, j]
            else:
                _n, o4, w4, _sub = rec
                a = o4
                while w4 > 0:
                    ri = a // 4
                    rows_sum[:, REST[ri][1]] += rs[:, j]
                    break  # pieces never cross region boundaries
        per_row = rows_sum.T.reshape(RPC)
        lc = lens[rows[c]].astype(np.float64)
        total += np.sum(per_row / lc)
    return np.array(total / B, dtype=np.float32)


def run(inputs, trace: bool = False):
    pred = np.asarray(inputs["pred"], dtype=np.float32)
    align = np.asarray(inputs["alignment"], dtype=np.float32)
    lens = np.asarray(inputs["token_lengths"])

    rows, W = _plan_sharding(lens)
    plan = _plan_layout(lens)
    nc = _get_module(plan)

    col = np.arange(T)[None, :]
    lcol = lens[:, None]
    pred_m = np.where(col < lcol, pred, 0.0).astype(np.float32)
    align_m = np.where(col < lcol, align, 1.0).astype(np.float32)

    in_maps = [{"data": _pack_core(pred_m, align_m, rows[c], plan["S"])}
               for c in range(N_CORES)]

    res = run_bass_kernel_spmd(nc, in_maps, core_ids=list(range(N_CORES)),
                               trace=trace)
    return _combine(res.results, lens, rows, plan), res


def kernel(**inputs) -> np.ndarray:
    out, _ = run(inputs, trace=False)
    return out


def _sim_module(lens):
    return _get_module(_plan_layout(np.asarray(lens)))


# revision 30
# speedup vs baseline: 1.0270x; 1.0007x over previous
"""Masked per-sample MSE loss (duration-predictor loss) on 8 Trainium2 cores.

Math (per the reference):
    mask[i, j]  = j < token_lengths[i]
    diff        = where(mask, pred - log(alignment), 0.0)
    out         = mean_i( sum_j diff[i,j]^2 / token_lengths[i] )

Sharding: data parallel over the batch dim with length-sorted row assignment.
Rows sorted by token_length; sorted rank r -> row-tile t = r // 1024, core
c = r % 8, partition p = (r % 1024) // 8. Tile t's rows all have length <=
W[t]; with S = max_t ceil(W[t]/(t+1)), tile t is treated as covering columns
[0, S*(t+1)).

Host-side packing: each core's pred and align rows are packed into ONE
[128, 20S] f32 DRAM buffer:
  [0,8S):    diag slots  [pd0 ad0 pd1 ad1 pd2 ad2 pd3 ad3], each S wide,
             where pd_t/ad_t = pred/align of tile t, cols [S*t, S*(t+1))
  [8S,14S):  pred rest   [t1[0,S) t2[0,S) t3[0,S) t2[S,2S) t3[S,2S) t3[2S,3S)]
  [14S,20S): align rest  (same region order)
Invalid positions are padded pred=0 / align=1 (ln 1 = 0), so the device
needs NO masks, iota, or length tensors.  The 8 diag slots have uniform
stride S, so ONE 3-dim DMA [p, slot, w] fetches the same column window of
both halves of all four tiles; sorted order lets diag chunks drop an exact
prefix of partitions — pure traffic pruning to ~52% of the dense bytes.
HWDGE descriptor generation is 625 ns serial per DMA, so DMA count is the
scarce resource: 14 loads cover everything, aligns in two big early slabs
and pred-rest in fine chunks so the tail chain stays short.

Device work: Ln(align) in place (ACT), d = pred - ln(align) in place (DVE or
Pool tensor_sub), then per-(piece, tile) square-with-row-sum-accum (DVE
scalar_tensor_tensor in place, ACT Square into scratch for a few big slices)
into a distinct rs column.  Compute always runs on all 128 partitions (the
ISA forbids partition-offset compute); rows below a pruned chunk's p0 leave
garbage in that chunk's rs column, which the host combine ignores.

Per-row divide by length and the global mean run on the host in float64.

Raw Bass with explicit semaphores (the walrus build rejects compute
instructions carrying more than one sync-wait, so waits are standalone)."""

from contextlib import ExitStack

import numpy as np

import concourse.bass as bass
from concourse import mybir
from concourse.bass_utils import run_bass_kernel_spmd

B, T = 4096, 2048
N_CORES = 8
RPC = B // N_CORES
P = 128
N_TILES = RPC // P
GROUP = P * N_CORES

F32 = mybir.dt.float32

_CACHE: dict = {}

# rest regions: (index, tile, col0_in_S_units)
REST = [(0, 1, 0), (1, 2, 0), (2, 3, 0), (3, 2, 1), (4, 3, 1), (5, 3, 2)]


def _plan_sharding(lens):
    order = np.argsort(lens, kind="stable")
    W = []
    for t in range(N_TILES):
        W.append(int(lens[order[t * GROUP:(t + 1) * GROUP]].max()))
    rows = []
    for c in range(N_CORES):
        ids = np.empty(RPC, dtype=np.int64)
        for t in range(N_TILES):
            ids[t * P:(t + 1) * P] = order[
                t * GROUP + c + N_CORES * np.arange(P)]
        rows.append(ids)
    return rows, W


def _plan_layout(lens):
    order = np.argsort(lens, kind="stable")
    slens = lens[order]
    W = [int(slens[t * GROUP:(t + 1) * GROUP].max()) for t in range(N_TILES)]
    S = max(-(-W[t] // (t + 1)) for t in range(N_TILES))
    S = max(512, -(-S // 4) * 4)

    mw = S // 4
    p0s = []
    for k in range(4):
        o = k * mw
        p0 = P
        for t in range(N_TILES):
            gl = slens[t * GROUP:(t + 1) * GROUP]
            cnt = int(np.searchsorted(gl, t * S + o, side="right"))
            p0 = min(p0, cnt // N_CORES)
        p0s.append(p0)
    return dict(S=S, p0s=p0s)


# ---------------------------------------------------------------------------
# Plan tables (iterated against TimelineSim).  Rest-piece coordinates are in
# S/4 units relative to the pred-rest window start (8S); m-chunk coordinates
# are in S/4 units within a diag slot.
#   DMA: ("cmK",) diag chunk K (both halves, all tiles, pruned)
#        (name, "a", o4, w4) align-rest slab   (o4 rel to align window)
#        (name, "p", o4, w4) pred-rest chunk   (o4 rel to pred window)
DMA_CHUNKS = [
    ("ar1", "a", 0, 12), ("cm0",), ("pc4a", "p", 0, 4), ("cm1",),
    ("pc4b", "p", 4, 4), ("cm2",), ("cm3",), ("ar2", "a", 12, 12),
    ("pc5a", "p", 8, 4), ("pc5b", "p", 12, 4), ("pc6", "p", 16, 4),
    ("pc7a", "p", 20, 2), ("pc7b", "p", 22, 1), ("pc8", "p", 23, 1),
]
# Ln: ("lmK",) diag chunk K (align slots), or (name, o4, w4, dep) rest piece
LN_OPS = [
    ("lm0",), ("lm1",), ("lm2",), ("lm3",),
    ("lr0", 0, 4, "ar1"), ("lr1", 4, 4, "ar1"), ("lr2", 8, 4, "ar1"),
    ("lr3", 12, 4, "ar2"), ("lr4", 16, 4, "ar2"), ("lr5", 20, 4, "ar2"),
]
# sub: ("smK",) diag chunk K, or (name, o4, w4, pred chunk, ln dep)
SUB_OPS = [
    ("sm0",), ("sm1",), ("sm2",), ("sm3",),
    ("sc4a", 0, 4, "pc4a", "lr0"), ("sc4b", 4, 4, "pc4b", "lr1"),
    ("sc5a", 8, 4, "pc5a", "lr2"), ("sc5b", 12, 4, "pc5b", "lr3"),
    ("sc6", 16, 4, "pc6", "lr4"),
    # c7a/c7b/c8 are all tile-3 cols [2S,3S): one sub once pc8 lands (DMA
    # rings are FIFO, so pc8's sem implies the earlier chunks are resident)
    ("sc78", 20, 4, "pc8", "lr5"),
]
# sq: ("qmKWtT", K, W, T) diag cols [K*q,(K+W)*q) of tile T, or
#     (name, o4, w4, sub dep) rest piece
SQ_OPS = (
    [(f"qm{k}t{t}", k, 1, t) for k in range(4) for t in range(4)]
    + [("qc4a", 0, 4, "sc4a"), ("qc4b", 4, 4, "sc4b"),
       ("qc5a", 8, 4, "sc5a"), ("qc5b", 12, 4, "sc5b"),
       ("qc6", 16, 4, "sc6"), ("qc78", 20, 4, "sc78")]
)
SUB_ENG = {"sc4a": "pool", "sc4b": "pool"}              # default dve
SQ_ENG = {"qc4a": "act", "qc4b": "act", "qc5a": "act", "qc5b": "act",
          "qc6": "act"}                                 # default dve
ACT_ORDER = ["lr0", "lr1", "lm0", "lr2", "lm1", "qc4a", "lm2", "lm3",
             "qc4b", "lr3", "lr4", "lr5", "qc5a", "qc5b", "qc6"]
DVE_ORDER = ["sm0", "qm0t0", "qm0t1", "qm0t2", "qm0t3",
             "sm1", "qm1t0", "qm1t1", "qm1t2", "qm1t3",
             "sm2", "qm2t0", "qm2t1", "qm2t2", "qm2t3",
             "sm3", "qm3t0", "qm3t1", "qm3t2", "qm3t3",
             "sc5a", "sc5b", "sc6", "sc78", "qc78"]
POOL_ORDER = ["sc4a", "sc4b"]

MSUB_CHUNK = {f"sm{k}": f"cm{k}" for k in range(4)}
MLN_CHUNK = {f"lm{k}": f"cm{k}" for k in range(4)}


def _build_module(plan):
    S = plan["S"]
    q = S // 4
    PW = 8 * S            # pred-rest window start
    AW = 14 * S           # align-rest window start
    NSQ = len(SQ_OPS)

    nc = bass.Bass("TRN2")
    data_d = nc.dram_tensor("data", [P, 20 * S], F32, kind="ExternalInput")
    out_d = nc.dram_tensor("rowsums", [P, NSQ], F32, kind="ExternalOutput")

    with ExitStack() as ctx:
        db = ctx.enter_context(nc.sbuf_tensor("db", [P, 20 * S], F32))
        sq_sb = ctx.enter_context(nc.sbuf_tensor("sq_sb", [P, 2, S], F32))
        rs_sb = ctx.enter_context(nc.sbuf_tensor("rs_sb", [P, NSQ], F32))
        s_c = {c[0]: ctx.enter_context(nc.semaphore(f"s_{c[0]}"))
               for c in DMA_CHUNKS}
        s_ln = ctx.enter_context(nc.semaphore("s_ln"))
        s_subv = ctx.enter_context(nc.semaphore("s_subv"))
        s_subp = ctx.enter_context(nc.semaphore("s_subp"))
        s_sqa = ctx.enter_context(nc.semaphore("s_sqa"))
        s_sqv = ctx.enter_context(nc.semaphore("s_sqv"))
        s_out = ctx.enter_context(nc.semaphore("s_out"))
        s_ms = ctx.enter_context(nc.semaphore("s_ms"))
        block = ctx.enter_context(nc.Block())

        def slots(x, p0, k, half=None):
            """Diag slot view, chunk k. half: None=all 8, 0=pred, 1=align."""
            v = x[p0:, :8 * S].rearrange("p (s w) -> p s w", s=8)
            v = v[:, :, k * q:(k + 1) * q]
            if half is None:
                return v
            return v[:, half::2, :]

        sub_count = {}
        seq = {"dve": 0, "pool": 0}
        for order in (DVE_ORDER, POOL_ORDER):
            for op in order:
                if op.startswith("s") and not op.startswith("sq"):
                    e = SUB_ENG.get(op, "dve")
                    seq[e] += 1
                    sub_count[op] = (e, seq[e])
        ln_pos = {}
        pos = 0
        for op in ACT_ORDER:
            if op.startswith("l"):
                pos += 1
                ln_pos[op] = pos

        @block.sync
        def _(sync):
            for c in DMA_CHUNKS:
                name = c[0]
                if len(c) == 1:  # diag chunk
                    k = int(name[2])
                    p0 = plan["p0s"][k]
                    sync.dma_start(
                        slots(db, p0, k), slots(data_d, p0, k),
                    ).then_inc(s_c[name], 16)
                else:
                    _n, half, o4, w4 = c
                    base = AW if half == "a" else PW
                    a, b = base + o4 * q, base + (o4 + w4) * q
                    sync.dma_start(
                        db[:, a:b], data_d[:, a:b],
                    ).then_inc(s_c[name], 16)
            n_sqa = sum(1 for s in SQ_OPS if SQ_ENG.get(s[0], "dve") == "act")
            if n_sqa:
                sync.wait_ge(s_sqa, n_sqa)
            if NSQ - n_sqa:
                sync.wait_ge(s_sqv, NSQ - n_sqa)
            sync.dma_start(out_d[:, :], rs_sb[:, :]).then_inc(s_out, 16)

        state = {"nasq": 0}

        def emit_sub(eng_obj, eng_name, op_rec):
            op = op_rec[0]
            if len(op_rec) == 1:  # diag
                k = int(op[2])
                eng_obj.wait_ge(s_c[MSUB_CHUNK[op]], 16)
                eng_obj.wait_ge(s_ln, ln_pos[f"lm{k}"])
                pv, av = slots(db, 0, k, 0), slots(db, 0, k, 1)
            else:
                _n, o4, w4, pch, lndep = op_rec
                eng_obj.wait_ge(s_c[pch], 16)
                eng_obj.wait_ge(s_ln, ln_pos[lndep])
                pv = db[:, PW + o4 * q:PW + (o4 + w4) * q]
                av = db[:, AW + o4 * q:AW + (o4 + w4) * q]
            eng_obj.tensor_sub(pv, pv, av).then_inc(
                s_subv if eng_name == "dve" else s_subp, 1)

        def emit_sq(eng_obj, eng_name, op, emitted_subv):
            rec = next(s for s in SQ_OPS if s[0] == op)
            rs_col = SQ_OPS.index(rec)
            if len(rec) == 4 and isinstance(rec[3], int):  # diag per-tile
                _n, k, kw, t = rec
                subdep = f"sm{k + kw - 1}"
                d = db[:, 2 * t * S + k * q:2 * t * S + (k + kw) * q]
                w = kw * q
            else:
                _n, o4, w4, subdep = rec
                d = db[:, PW + o4 * q:PW + (o4 + w4) * q]
                w = w4 * q
            se, cnt = sub_count[subdep]
            if eng_name != se or (eng_name == "dve" and cnt > emitted_subv):
                eng_obj.wait_ge(s_subv if se == "dve" else s_subp, cnt)
            if eng_name == "act":
                if state["nasq"] >= 2:
                    eng_obj.wait_ge(s_sqa, state["nasq"] - 1)
                eng_obj.activation(
                    sq_sb[:, state["nasq"] % 2, :w], d,
                    mybir.ActivationFunctionType.Square,
                    accum_out=rs_sb[:, rs_col:rs_col + 1],
                ).then_inc(s_sqa, 1)
                state["nasq"] += 1
            else:
                eng_obj.scalar_tensor_tensor(
                    out=d, in0=d, scalar=1.0, in1=d,
                    op0=mybir.AluOpType.mult, op1=mybir.AluOpType.mult,
                    accum_out=rs_sb[:, rs_col:rs_col + 1],
                ).then_inc(s_sqv, 1)

        @block.scalar
        def _(scalar):
            for op in ACT_ORDER:
                if op.startswith("l"):
                    rec = next(l for l in LN_OPS if l[0] == op)
                    if len(rec) == 1:
                        k = int(op[2])
                        scalar.wait_ge(s_c[MLN_CHUNK[op]], 16)
                        ap = slots(db, 0, k, 1)
                    else:
                        _n, o4, w4, dep = rec
                        scalar.wait_ge(s_c[dep], 16)
                        ap = db[:, AW + o4 * q:AW + (o4 + w4) * q]
                    scalar.activation(
                        ap, ap, mybir.ActivationFunctionType.Ln,
                    ).then_inc(s_ln, 1)
                else:
                    emit_sq(scalar, "act", op, 0)

        @block.vector
        def _(v):
            emitted = 0
            for op in DVE_ORDER:
                if op.startswith("sm") or op.startswith("sc"):
                    emit_sub(v, "dve", next(s for s in SUB_OPS if s[0] == op))
                    emitted += 1
                else:
                    emit_sq(v, "dve", op, emitted)

        @block.gpsimd
        def _(g):
            for op in POOL_ORDER:
                emit_sub(g, "pool", next(s for s in SUB_OPS if s[0] == op))

    return nc


def _get_module(plan):
    key = (plan["S"], tuple(plan["p0s"]))
    if key not in _CACHE:
        _CACHE[key] = _build_module(plan)
    return _CACHE[key]


def _pack_core(pred_m, align_m, ids, S):
    buf = np.empty((P, 20 * S), dtype=np.float32)
    buf[:, :8 * S:2 * S] = 0.0  # harmless init; fully overwritten below
    ids_t = ids.reshape(N_TILES, P)
    for t in range(N_TILES):
        col0 = t * S
        w = min(S, T - col0)
        pd = np.zeros((P, S), dtype=np.float32)
        ad = np.ones((P, S), dtype=np.float32)
        if w > 0:
            pd[:, :w] = pred_m[ids_t[t], col0:col0 + w]
            ad[:, :w] = align_m[ids_t[t], col0:col0 + w]
        buf[:, 2 * t * S:(2 * t + 1) * S] = pd
        buf[:, (2 * t + 1) * S:(2 * t + 2) * S] = ad
    for i, t, c0 in REST:
        col0 = c0 * S
        w = min(S, T - col0)
        pr = np.zeros((P, S), dtype=np.float32)
        ar = np.ones((P, S), dtype=np.float32)
        if w > 0:
            pr[:, :w] = pred_m[ids_t[t], col0:col0 + w]
            ar[:, :w] = align_m[ids_t[t], col0:col0 + w]
        buf[:, (8 + i) * S:(9 + i) * S] = pr
        buf[:, (14 + i) * S:(15 + i) * S] = ar
    return buf


def _combine(results, lens, rows, plan):
    total = 0.0
    p0s = plan["p0s"]
    for c in range(N_CORES):
        rs = np.asarray(results[c]["rowsums"], dtype=np.float64)
        rows_sum = np.zeros((P, N_TILES))
        for j, rec in enumerate(SQ_OPS):
            if len(rec) == 4 and isinstance(rec[3], int):
                _n, k, _kw, t = rec
                rows_sum[p0s[k]:, t] += rs[p0s[k]:, j]
            else:
                _n, o4, w4, _sub = rec
                a = o4
                while w4 > 0:
                    ri = a // 4
                    rows_sum[:, REST[ri][1]] += rs[:, j]
                    break  # pieces never cross region boundaries
        per_row = rows_sum.T.reshape(RPC)
        lc = lens[rows[c]].astype(np.float64)
        total += np.sum(per_row / lc)
    return np.array(total / B, dtype=np.float32)


def run(inputs, trace: bool = False):
    pred = np.asarray(inputs["pred"], dtype=np.float32)
    align = np.asarray(inputs["alignment"], dtype=np.float32)
    lens = np.asarray(inputs["token_lengths"])

    rows, W = _plan_sharding(lens)
    plan = _plan_layout(lens)
    nc = _get_module(plan)

    col = np.arange(T)[None, :]
    lcol = lens[:, None]
    pred_m = np.where(col < lcol, pred, 0.0).astype(np.float32)
    align_m = np.where(col < lcol, align, 1.0).astype(np.float32)

    in_maps = [{"data": _pack_core(pred_m, align_m, rows[c], plan["S"])}
               for c in range(N_CORES)]

    res = run_bass_kernel_spmd(nc, in_maps, core_ids=list(range(N_CORES)),
                               trace=trace)
    return _combine(res.results, lens, rows, plan), res


def kernel(**inputs) -> np.ndarray:
    out, _ = run(inputs, trace=False)
    return out


def _sim_module(lens):
    return _get_module(_plan_layout(np.asarray(lens)))


# revision 31
# speedup vs baseline: 1.0289x; 1.0018x over previous
"""Masked per-sample MSE loss (duration-predictor loss) on 8 Trainium2 cores.

Math (per the reference):
    mask[i, j]  = j < token_lengths[i]
    diff        = where(mask, pred - log(alignment), 0.0)
    out         = mean_i( sum_j diff[i,j]^2 / token_lengths[i] )

Sharding: data parallel over the batch dim with length-sorted row assignment.
Rows sorted by token_length; sorted rank r -> row-tile t = r // 1024, core
c = r % 8, partition p = (r % 1024) // 8. Tile t's rows all have length <=
W[t]; with S = max_t ceil(W[t]/(t+1)), tile t is treated as covering columns
[0, S*(t+1)).

Host-side packing: each core's pred and align rows are packed into ONE
[128, 20S] f32 DRAM buffer:
  [0,8S):    diag slots  [pd0 ad0 pd1 ad1 pd2 ad2 pd3 ad3], each S wide,
             where pd_t/ad_t = pred/align of tile t, cols [S*t, S*(t+1))
  [8S,14S):  pred rest   [t1[0,S) t2[0,S) t3[0,S) t2[S,2S) t3[S,2S) t3[2S,3S)]
  [14S,20S): align rest  (same region order)
Invalid positions are padded pred=0 / align=1 (ln 1 = 0), so the device
needs NO masks, iota, or length tensors.  The 8 diag slots have uniform
stride S, so ONE 3-dim DMA [p, slot, w] fetches the same column window of
both halves of all four tiles; sorted order lets diag chunks drop an exact
prefix of partitions — pure traffic pruning to ~52% of the dense bytes.
HWDGE descriptor generation is 625 ns serial per DMA, so DMA count is the
scarce resource: 14 loads cover everything, aligns in two big early slabs
and pred-rest in fine chunks so the tail chain stays short.

Device work: Ln(align) in place (ACT), d = pred - ln(align) in place (DVE or
Pool tensor_sub), then per-(piece, tile) square-with-row-sum-accum (DVE
scalar_tensor_tensor in place, ACT Square into scratch for a few big slices)
into a distinct rs column.  Compute always runs on all 128 partitions (the
ISA forbids partition-offset compute); rows below a pruned chunk's p0 leave
garbage in that chunk's rs column, which the host combine ignores.

Per-row divide by length and the global mean run on the host in float64.

Raw Bass with explicit semaphores (the walrus build rejects compute
instructions carrying more than one sync-wait, so waits are standalone)."""

from contextlib import ExitStack

import numpy as np

import concourse.bass as bass
from concourse import mybir
from concourse.bass_utils import run_bass_kernel_spmd

B, T = 4096, 2048
N_CORES = 8
RPC = B // N_CORES
P = 128
N_TILES = RPC // P
GROUP = P * N_CORES

F32 = mybir.dt.float32

_CACHE: dict = {}

# rest regions: (index, tile, col0_in_S_units)
REST = [(0, 1, 0), (1, 2, 0), (2, 3, 0), (3, 2, 1), (4, 3, 1), (5, 3, 2)]


def _plan_sharding(lens):
    order = np.argsort(lens, kind="stable")
    W = []
    for t in range(N_TILES):
        W.append(int(lens[order[t * GROUP:(t + 1) * GROUP]].max()))
    rows = []
    for c in range(N_CORES):
        ids = np.empty(RPC, dtype=np.int64)
        for t in range(N_TILES):
            ids[t * P:(t + 1) * P] = order[
                t * GROUP + c + N_CORES * np.arange(P)]
        rows.append(ids)
    return rows, W


def _plan_layout(lens):
    order = np.argsort(lens, kind="stable")
    slens = lens[order]
    W = [int(slens[t * GROUP:(t + 1) * GROUP].max()) for t in range(N_TILES)]
    S = max(-(-W[t] // (t + 1)) for t in range(N_TILES))
    S = max(512, -(-S // 4) * 4)

    mw = S // 4
    p0s = []
    for k in range(4):
        o = k * mw
        p0 = P
        for t in range(N_TILES):
            gl = slens[t * GROUP:(t + 1) * GROUP]
            cnt = int(np.searchsorted(gl, t * S + o, side="right"))
            p0 = min(p0, cnt // N_CORES)
        p0s.append(p0)
    return dict(S=S, p0s=p0s)


# ---------------------------------------------------------------------------
# Plan tables (iterated against TimelineSim).  Rest-piece coordinates are in
# S/4 units relative to the pred-rest window start (8S); m-chunk coordinates
# are in S/4 units within a diag slot.
#   DMA: ("cmK",) diag chunk K (both halves, all tiles, pruned)
#        (name, "a", o4, w4) align-rest slab   (o4 rel to align window)
#        (name, "p", o4, w4) pred-rest chunk   (o4 rel to pred window)
DMA_CHUNKS = [
    ("ar1", "a", 0, 12), ("cm0",), ("pc4a", "p", 0, 4), ("cm1",),
    ("pc4b", "p", 4, 4), ("cm2",), ("cm3",), ("ar2", "a", 12, 12),
    ("pc5a", "p", 8, 4), ("pc5b", "p", 12, 4), ("pc6", "p", 16, 4),
    ("pc7a", "p", 20, 2), ("pc7b", "p", 22, 1), ("pc8", "p", 23, 1),
]
# Ln: ("lmK",) diag chunk K (align slots), or (name, o4, w4, dep) rest piece
LN_OPS = [
    ("lm0",), ("lm1",), ("lm2",), ("lm3",),
    ("lr0", 0, 4, "ar1"), ("lr1", 4, 4, "ar1"), ("lr2", 8, 4, "ar1"),
    ("lr3", 12, 4, "ar2"), ("lr4", 16, 4, "ar2"), ("lr5", 20, 4, "ar2"),
]
# sub: ("smK",) diag chunk K, or (name, o4, w4, pred chunk, ln dep)
SUB_OPS = [
    ("sm0",), ("sm1",), ("sm2",), ("sm3",),
    ("sc4a", 0, 4, "pc4a", "lr0"), ("sc4b", 4, 4, "pc4b", "lr1"),
    ("sc5a", 8, 4, "pc5a", "lr2"), ("sc5b", 12, 4, "pc5b", "lr3"),
    ("sc6", 16, 4, "pc6", "lr4"),
    # c7a/c7b/c8 are all tile-3 cols [2S,3S): one sub once pc8 lands (DMA
    # rings are FIFO, so pc8's sem implies the earlier chunks are resident)
    ("sc78", 20, 4, "pc8", "lr5"),
]
# sq: ("qmKWtT", K, W, T) diag cols [K*q,(K+W)*q) of tile T, or
#     (name, o4, w4, sub dep) rest piece
SQ_OPS = (
    [(f"qm{k}t{t}", k, 1, t) for k in range(4) for t in range(4)]
    + [("qc4a", 0, 4, "sc4a"), ("qc4b", 4, 4, "sc4b"),
       ("qc5a", 8, 4, "sc5a"), ("qc5b", 12, 4, "sc5b"),
       ("qc6", 16, 3, "sc6"), ("qc6b", 19, 1, "sc6"),
       ("qc78", 20, 4, "sc78")]
)
SUB_ENG = {"sc4a": "pool", "sc4b": "pool"}              # default dve
SQ_ENG = {"qc4a": "act", "qc4b": "act", "qc5a": "act", "qc5b": "act",
          "qc6": "act"}                                 # default dve
ACT_ORDER = ["lr0", "lr1", "lm0", "lr2", "lm1", "qc4a", "lm2", "lm3",
             "qc4b", "lr3", "lr4", "lr5", "qc5a", "qc5b", "qc6"]
DVE_ORDER = ["sm0", "qm0t0", "qm0t1", "qm0t2", "qm0t3",
             "sm1", "qm1t0", "qm1t1", "qm1t2", "qm1t3",
             "sm2", "qm2t0", "qm2t1", "qm2t2", "qm2t3",
             "sm3", "qm3t0", "qm3t1", "qm3t2", "qm3t3",
             "sc5a", "sc5b", "sc6", "qc6b", "sc78", "qc78"]
POOL_ORDER = ["sc4a", "sc4b"]

MSUB_CHUNK = {f"sm{k}": f"cm{k}" for k in range(4)}
MLN_CHUNK = {f"lm{k}": f"cm{k}" for k in range(4)}


def _build_module(plan):
    S = plan["S"]
    q = S // 4
    PW = 8 * S            # pred-rest window start
    AW = 14 * S           # align-rest window start
    NSQ = len(SQ_OPS)

    nc = bass.Bass("TRN2")
    data_d = nc.dram_tensor("data", [P, 20 * S], F32, kind="ExternalInput")
    out_d = nc.dram_tensor("rowsums", [P, NSQ], F32, kind="ExternalOutput")

    with ExitStack() as ctx:
        db = ctx.enter_context(nc.sbuf_tensor("db", [P, 20 * S], F32))
        sq_sb = ctx.enter_context(nc.sbuf_tensor("sq_sb", [P, 2, S], F32))
        rs_sb = ctx.enter_context(nc.sbuf_tensor("rs_sb", [P, NSQ], F32))
        s_c = {c[0]: ctx.enter_context(nc.semaphore(f"s_{c[0]}"))
               for c in DMA_CHUNKS}
        s_ln = ctx.enter_context(nc.semaphore("s_ln"))
        s_subv = ctx.enter_context(nc.semaphore("s_subv"))
        s_subp = ctx.enter_context(nc.semaphore("s_subp"))
        s_sqa = ctx.enter_context(nc.semaphore("s_sqa"))
        s_sqv = ctx.enter_context(nc.semaphore("s_sqv"))
        s_out = ctx.enter_context(nc.semaphore("s_out"))
        s_ms = ctx.enter_context(nc.semaphore("s_ms"))
        block = ctx.enter_context(nc.Block())

        def slots(x, p0, k, half=None):
            """Diag slot view, chunk k. half: None=all 8, 0=pred, 1=align."""
            v = x[p0:, :8 * S].rearrange("p (s w) -> p s w", s=8)
            v = v[:, :, k * q:(k + 1) * q]
            if half is None:
                return v
            return v[:, half::2, :]

        sub_count = {}
        seq = {"dve": 0, "pool": 0}
        for order in (DVE_ORDER, POOL_ORDER):
            for op in order:
                if op.startswith("s") and not op.startswith("sq"):
                    e = SUB_ENG.get(op, "dve")
                    seq[e] += 1
                    sub_count[op] = (e, seq[e])
        ln_pos = {}
        pos = 0
        for op in ACT_ORDER:
            if op.startswith("l"):
                pos += 1
                ln_pos[op] = pos

        @block.sync
        def _(sync):
            for c in DMA_CHUNKS:
                name = c[0]
                if len(c) == 1:  # diag chunk
                    k = int(name[2])
                    p0 = plan["p0s"][k]
                    sync.dma_start(
                        slots(db, p0, k), slots(data_d, p0, k),
                    ).then_inc(s_c[name], 16)
                else:
                    _n, half, o4, w4 = c
                    base = AW if half == "a" else PW
                    a, b = base + o4 * q, base + (o4 + w4) * q
                    sync.dma_start(
                        db[:, a:b], data_d[:, a:b],
                    ).then_inc(s_c[name], 16)
            n_sqa = sum(1 for s in SQ_OPS if SQ_ENG.get(s[0], "dve") == "act")
            if n_sqa:
                sync.wait_ge(s_sqa, n_sqa)
            if NSQ - n_sqa:
                sync.wait_ge(s_sqv, NSQ - n_sqa)
            sync.dma_start(out_d[:, :], rs_sb[:, :]).then_inc(s_out, 16)

        state = {"nasq": 0}

        def emit_sub(eng_obj, eng_name, op_rec):
            op = op_rec[0]
            if len(op_rec) == 1:  # diag
                k = int(op[2])
                eng_obj.wait_ge(s_c[MSUB_CHUNK[op]], 16)
                eng_obj.wait_ge(s_ln, ln_pos[f"lm{k}"])
                pv, av = slots(db, 0, k, 0), slots(db, 0, k, 1)
            else:
                _n, o4, w4, pch, lndep = op_rec
                eng_obj.wait_ge(s_c[pch], 16)
                eng_obj.wait_ge(s_ln, ln_pos[lndep])
                pv = db[:, PW + o4 * q:PW + (o4 + w4) * q]
                av = db[:, AW + o4 * q:AW + (o4 + w4) * q]
            eng_obj.tensor_sub(pv, pv, av).then_inc(
                s_subv if eng_name == "dve" else s_subp, 1)

        def emit_sq(eng_obj, eng_name, op, emitted_subv):
            rec = next(s for s in SQ_OPS if s[0] == op)
            rs_col = SQ_OPS.index(rec)
            if len(rec) == 4 and isinstance(rec[3], int):  # diag per-tile
                _n, k, kw, t = rec
                subdep = f"sm{k + kw - 1}"
                d = db[:, 2 * t * S + k * q:2 * t * S + (k + kw) * q]
                w = kw * q
            else:
                _n, o4, w4, subdep = rec
                d = db[:, PW + o4 * q:PW + (o4 + w4) * q]
                w = w4 * q
            se, cnt = sub_count[subdep]
            if eng_name != se or (eng_name == "dve" and cnt > emitted_subv):
                eng_obj.wait_ge(s_subv if se == "dve" else s_subp, cnt)
            if eng_name == "act":
                if state["nasq"] >= 2:
                    eng_obj.wait_ge(s_sqa, state["nasq"] - 1)
                eng_obj.activation(
                    sq_sb[:, state["nasq"] % 2, :w], d,
                    mybir.ActivationFunctionType.Square,
                    accum_out=rs_sb[:, rs_col:rs_col + 1],
                ).then_inc(s_sqa, 1)
                state["nasq"] += 1
            else:
                eng_obj.scalar_tensor_tensor(
                    out=d, in0=d, scalar=1.0, in1=d,
                    op0=mybir.AluOpType.mult, op1=mybir.AluOpType.mult,
                    accum_out=rs_sb[:, rs_col:rs_col + 1],
                ).then_inc(s_sqv, 1)

        @block.scalar
        def _(scalar):
            for op in ACT_ORDER:
                if op.startswith("l"):
                    rec = next(l for l in LN_OPS if l[0] == op)
                    if len(rec) == 1:
                        k = int(op[2])
                        scalar.wait_ge(s_c[MLN_CHUNK[op]], 16)
                        ap = slots(db, 0, k, 1)
                    else:
                        _n, o4, w4, dep = rec
                        scalar.wait_ge(s_c[dep], 16)
                        ap = db[:, AW + o4 * q:AW + (o4 + w4) * q]
                    scalar.activation(
                        ap, ap, mybir.ActivationFunctionType.Ln,
                    ).then_inc(s_ln, 1)
                else:
                    emit_sq(scalar, "act", op, 0)

        @block.vector
        def _(v):
            emitted = 0
            for op in DVE_ORDER:
                if op.startswith("sm") or op.startswith("sc"):
                    emit_sub(v, "dve", next(s for s in SUB_OPS if s[0] == op))
                    emitted += 1
                else:
                    emit_sq(v, "dve", op, emitted)

        @block.gpsimd
        def _(g):
            for op in POOL_ORDER:
                emit_sub(g, "pool", next(s for s in SUB_OPS if s[0] == op))

    return nc


def _get_module(plan):
    key = (plan["S"], tuple(plan["p0s"]))
    if key not in _CACHE:
        _CACHE[key] = _build_module(plan)
    return _CACHE[key]


def _pack_core(pred_m, align_m, ids, S):
    buf = np.empty((P, 20 * S), dtype=np.float32)
    buf[:, :8 * S:2 * S] = 0.0  # harmless init; fully overwritten below
    ids_t = ids.reshape(N_TILES, P)
    for t in range(N_TILES):
        col0 = t * S
        w = min(S, T - col0)
        pd = np.zeros((P, S), dtype=np.float32)
        ad = np.ones((P, S), dtype=np.float32)
        if w > 0:
            pd[:, :w] = pred_m[ids_t[t], col0:col0 + w]
            ad[:, :w] = align_m[ids_t[t], col0:col0 + w]
        buf[:, 2 * t * S:(2 * t + 1) * S] = pd
        buf[:, (2 * t + 1) * S:(2 * t + 2) * S] = ad
    for i, t, c0 in REST:
        col0 = c0 * S
        w = min(S, T - col0)
        pr = np.zeros((P, S), dtype=np.float32)
        ar = np.ones((P, S), dtype=np.float32)
        if w > 0:
            pr[:, :w] = pred_m[ids_t[t], col0:col0 + w]
            ar[:, :w] = align_m[ids_t[t], col0:col0 + w]
        buf[:, (8 + i) * S:(9 + i) * S] = pr
        buf[:, (14 + i) * S:(15 + i) * S] = ar
    return buf


def _combine(results, lens, rows, plan):
    total = 0.0
    p0s = plan["p0s"]
    for c in range(N_CORES):
        rs = np.asarray(results[c]["rowsums"], dtype=np.float64)
        rows_sum = np.zeros((P, N_TILES))
        for j, rec in enumerate(SQ_OPS):
            if len(rec) == 4 and isinstance(rec[3], int):
                _n, k, _kw, t = rec
                rows_sum[p0s[k]:, t] += rs[p0s[k]:, j]
            else:
                _n, o4, w4, _sub = rec
                a = o4
                while w4 > 0:
                    ri = a // 4
                    rows_sum[:, REST[ri][1]] += rs[:, j]
                    break  # pieces never cross region boundaries
        per_row = rows_sum.T.reshape(RPC)
        lc = lens[rows[c]].astype(np.float64)
        total += np.sum(per_row / lc)
    return np.array(total / B, dtype=np.float32)


def run(inputs, trace: bool = False):
    pred = np.asarray(inputs["pred"], dtype=np.float32)
    align = np.asarray(inputs["alignment"], dtype=np.float32)
    lens = np.asarray(inputs["token_lengths"])

    rows, W = _plan_sharding(lens)
    plan = _plan_layout(lens)
    nc = _get_module(plan)

    col = np.arange(T)[None, :]
    lcol = lens[:, None]
    pred_m = np.where(col < lcol, pred, 0.0).astype(np.float32)
    align_m = np.where(col < lcol, align, 1.0).astype(np.float32)

    in_maps = [{"data": _pack_core(pred_m, align_m, rows[c], plan["S"])}
               for c in range(N_CORES)]

    res = run_bass_kernel_spmd(nc, in_maps, core_ids=list(range(N_CORES)),
                               trace=trace)
    return _combine(res.results, lens, rows, plan), res


def kernel(**inputs) -> np.ndarray:
    out, _ = run(inputs, trace=False)
    return out


def _sim_module(lens):
    return _get_module(_plan_layout(np.asarray(lens)))
